# revision 1
# baseline (speedup 1.0000x reference)
"""AlphaCompositor Trainium2 kernel.

out[n,c,h,w] = sum_k w[n,k,h,w] * ptclds[c, fragments[n,k,h,w]]
  w = alpha * prod_{j<k}(1 - alpha_j), invalid (-1) fragments contribute 0.

Strategy: data-parallel over N (8 cores). Per core the dominant cost is the
random gather of ~1M x 256B table rows, done with the GPSIMD dma_gather unit
(int16 indices, <=1024 per instruction). Indices are signed-shifted against
two window bases so the full 100K-row table is addressable in two gather
rounds; out-of-window slots redirect to embedded zero rows. Slots are laid
out in a padded space (63 pixels = 1008 real + 16 pad slots per 1024-slot
tile) so the last index of every gather is a pad (>=0) and the ucode's
trailing-negative truncation never fires. Weights are computed on-device via
Ln -> block-triangular PE matmul (exclusive cumsum over K) -> Exp, applied
with a broadcast-AP multiply, and reduced over K by PE matmuls accumulating
into PSUM.
"""

import sys
import types

import numpy as np

_N, _K, _H, _W = 8, 16, 256, 256
_C, _P = 64, 100000
_HWPIX = _H * _W                  # 65536 pixels / core
_PIX_T = 63                       # real pixels per gather tile
_NI = 1024                        # slots per gather instruction (ucode max)
_GT = 16                          # tiles per block
_NBLK = 66                        # blocks (ceil(ceil(65536/63)/16))
_NT = _NBLK * _GT                 # 1056 tiles (incl. pad tiles)
_SLOTP = _NT * _NI                # padded slot count
_BASE_A = 32768                   # window A: dev rows [0, 65536)
_SPLIT = 64512                    # orig rows [0, SPLIT) live in window A
_ZBASE_A = 64512                  # 1024 rotating zero rows in window A
_BASE_B = 69280                   # window B: dev rows [36512, 102048)
_ZBASE_B = 101024                 # 1024 rotating zero rows in window B
_PT_DEV = 102048


def _install_axon_shim():
    """Provide antenv.axon_hooks (missing on this image) and register the
    NTFF profile hook so trace=True yields exec_time_ns under axon."""
    if "antenv.axon_hooks" in sys.modules:
        return
    mod = types.ModuleType("antenv.axon_hooks")
    mod._hook = None
    mod.set_axon_ntff_profile_hook = lambda h: setattr(mod, "_hook", h)
    mod.get_axon_ntff_profile_hook = lambda: mod._hook
    sys.modules["antenv.axon_hooks"] = mod
    try:
        import antenv

        antenv.axon_hooks = mod
        from trn_agent_boot.trn_boot import _ntff_profile_via_ctypes

        mod.set_axon_ntff_profile_hook(
            _ntff_profile_via_ctypes("/opt/axon/libaxon_pjrt.so")
        )
    except Exception:
        pass


_BUILT = None


def _build():
    global _BUILT
    if _BUILT is not None:
        return _BUILT
    if "/opt/trn_rl_repo" not in sys.path:
        sys.path.insert(0, "/opt/trn_rl_repo")
    _install_axon_shim()
    import concourse.bacc as bacc
    import concourse.mybir as mybir
    from concourse.tile import TileContext

    f32 = mybir.dt.float32
    i16 = mybir.dt.int16

    nc = bacc.Bacc(
        "TRN2",
        target_bir_lowering=False,
        debug=False,
        num_devices=_N,
        num_swdge_queues=4,
    )
    table = nc.dram_tensor("table", [_PT_DEV, _C], f32, kind="ExternalInput")
    alph = nc.dram_tensor("alph", [128, _SLOTP // 128], f32, kind="ExternalInput")
    idxa = nc.dram_tensor("idxa", [_NBLK, 128, _GT * (_NI // 16)], i16, kind="ExternalInput")
    idxb = nc.dram_tensor("idxb", [_NBLK, 128, _GT * (_NI // 16)], i16, kind="ExternalInput")
    tri = nc.dram_tensor("tri", [128, 128], f32, kind="ExternalInput")
    sum16 = nc.dram_tensor("sum16", [128, 16 * 128], f32, kind="ExternalInput")
    out = nc.dram_tensor("out", [_NBLK, 128, 512], f32, kind="ExternalOutput")

    qn = 0
    with TileContext(nc) as tc:
        with (
            tc.tile_pool(name="const", bufs=1) as constp,
            tc.tile_pool(name="wts", bufs=3) as wtsp,
            tc.tile_pool(name="idxp", bufs=3) as idxp,
            tc.tile_pool(name="gp", bufs=14) as gp,
            tc.tile_pool(name="wgp", bufs=8) as wgp,
            tc.tile_pool(name="stg", bufs=3) as stgp,
            tc.tile_pool(name="ps", bufs=2, space="PSUM") as psp,
            tc.tile_pool(name="pslt", bufs=2, space="PSUM") as psltp,
        ):
            tri_sb = constp.tile([128, 128], f32)
            nc.sync.dma_start(out=tri_sb[:], in_=tri[:])
            sum16_sb = constp.tile([128, 16 * 128], f32)
            nc.sync.dma_start(out=sum16_sb[:], in_=sum16[:])

            for blk in range(_NBLK):
                at = wtsp.tile([128, 128], f32, tag="alph")
                nc.sync.dma_start(
                    out=at[:], in_=alph[:, blk * 128 : (blk + 1) * 128]
                )
                l1m = wtsp.tile([128, 128], f32, tag="l1m")
                nc.scalar.activation(
                    l1m[:], at[:], mybir.ActivationFunctionType.Ln,
                    bias=1.0, scale=-1.0,
                )
                ps_lt = psltp.tile([128, 128], f32)
                nc.tensor.matmul(
                    ps_lt[:], lhsT=tri_sb[:], rhs=l1m[:], start=True, stop=True
                )
                tex = wtsp.tile([128, 128], f32, tag="tex")
                nc.scalar.activation(
                    tex[:], ps_lt[:], mybir.ActivationFunctionType.Exp
                )
                wt = wtsp.tile([128, 128], f32, tag="wt")
                nc.vector.tensor_mul(out=wt[:], in0=at[:], in1=tex[:])

                ps_o = psp.tile([128, 512], f32)
                icols = _NI // 16
                tib = _GT if blk < _NBLK - 1 else 1  # last block: 1 real tile
                it_blk = []
                for rnd, idxd in ((0, idxa), (1, idxb)):
                    itb = idxp.tile([128, _GT * icols], i16, tag=f"it{rnd}")
                    nc.sync.dma_start(out=itb[:], in_=idxd[blk])
                    it_blk.append(itb)
                for j in range(tib):
                    t = blk * _GT + j
                    for rnd, rowoff in ((0, _BASE_A), (1, _BASE_B)):
                        it = it_blk[rnd][:, j * icols : (j + 1) * icols]
                        g = gp.tile([128, _NI // 128, _C], f32)
                        nc.gpsimd.dma_gather(
                            g[:], table[rowoff:, :], it, _NI, _NI, _C,
                            queue_num=qn,
                        )
                        qn = (qn + 1) % 4
                        wg = wgp.tile([128, 8, _C], f32)
                        wslice = (
                            wt[:, 8 * j : 8 * (j + 1)]
                            .rearrange("p (b one) -> p b one", one=1)
                            .to_broadcast([128, 8, _C])
                        )
                        nc.vector.tensor_mul(
                            out=wg[:], in0=g[:, 0:8, :], in1=wslice
                        )
                        nc.tensor.matmul(
                            ps_o[:],
                            lhsT=sum16_sb[:, 128 * j : 128 * (j + 1)],
                            rhs=wg[:].rearrange("p b c -> p (b c)"),
                            start=(j == 0 and rnd == 0),
                            stop=(j == tib - 1 and rnd == 1),
                        )
                stage = stgp.tile([128, 512], f32)
                nc.vector.tensor_copy(out=stage[:], in_=ps_o[:])
                nc.sync.dma_start(out=out[blk], in_=stage[:])

    nc.compile()
    _BUILT = nc
    return nc


def _tri_np():
    p = np.arange(128)
    f = np.arange(128)
    blk = (p[:, None] // 16) == (f[None, :] // 16)
    low = (p[:, None] % 16) < (f[None, :] % 16)
    return (blk & low).astype(np.float32)


def _sum16_np():
    # S[j][p][f] = 1 iff f == 8*j + p//16;  shipped as [128, 16*128] (j-major)
    j = np.arange(16)[:, None, None]
    p = np.arange(128)[None, :, None]
    f = np.arange(128)[None, None, :]
    s = (f == 8 * j + p // 16).astype(np.float32)  # [16, 128, 128]
    return np.ascontiguousarray(s.transpose(1, 0, 2).reshape(128, 16 * 128))


def _wrap_idx(idx_pad):
    """[SLOTP] int16 -> [NBLK, 128, GT*(NI//16)]: per-tile ucode 16-wrap
    (8x partition replication), tiles of a block packed along free dim."""
    w = idx_pad.reshape(_NT, _NI // 16, 16).transpose(0, 2, 1)  # [NT,16,IC]
    full = np.broadcast_to(
        w[:, None, :, :], (_NT, 8, 16, _NI // 16)
    ).reshape(_NT, 128, _NI // 16)
    return np.ascontiguousarray(
        full.reshape(_NBLK, _GT, 128, _NI // 16)
        .transpose(0, 2, 1, 3)
        .reshape(_NBLK, 128, _GT * (_NI // 16))
    )


# padded-slot position of real slot (pix, k): 1024*(pix//63) + 16*(pix%63) + k
_PIX = np.arange(_HWPIX)
_PPOS0 = 1024 * (_PIX // _PIX_T) + 16 * (_PIX % _PIX_T)  # k=0 position


def kernel(fragments, alphas, ptclds):
    nc = _build()
    from concourse.bass_utils import run_bass_kernel_spmd

    tbl = np.zeros((_PT_DEV, _C), np.float32)
    pt = np.ascontiguousarray(ptclds.T.astype(np.float32))  # [P, C]
    tbl[0:_SPLIT] = pt[0:_SPLIT]
    tbl[65536:101024] = pt[_SPLIT:_P]

    tri_np = _tri_np()
    sum16_np = _sum16_np()
    ppos = (_PPOS0[:, None] + np.arange(_K)[None, :]).reshape(-1)  # [HWPIX*K]

    in_maps = []
    for n in range(_N):
        f = fragments[n].reshape(_K, _HWPIX).T.reshape(-1).astype(np.int64)
        a = alphas[n].reshape(_K, _HWPIX).T.reshape(-1).astype(np.float32)
        valid = f >= 0
        a_m = np.where(valid, a, 0.0).astype(np.float32)

        # rotating zero rows, indexed by within-tile position, spread HBM banks
        zrot_full = (ppos % _NI) % 1024
        conda = valid & (f < _SPLIT)
        ia = np.where(
            conda, f - _BASE_A, _ZBASE_A + zrot_full - _BASE_A
        ).astype(np.int16)
        condb = valid & (f >= _SPLIT)
        ib = np.where(
            condb, f + 1024 - _BASE_B, _ZBASE_B + zrot_full - _BASE_B
        ).astype(np.int16)

        zrot_pad = (np.arange(_SLOTP) % _NI) % 1024
        a_p = np.zeros(_SLOTP, np.float32)
        ia_p = (_ZBASE_A + zrot_pad - _BASE_A).astype(np.int16)
        ib_p = (_ZBASE_B + zrot_pad - _BASE_B).astype(np.int16)
        a_p[ppos] = a_m
        ia_p[ppos] = ia
        ib_p[ppos] = ib

        in_maps.append(
            {
                "table": tbl,
                "alph": np.ascontiguousarray(a_p.reshape(-1, 128).T),
                "idxa": _wrap_idx(ia_p),
                "idxb": _wrap_idx(ib_p),
                "tri": tri_np,
                "sum16": sum16_np,
            }
        )

    res = run_bass_kernel_spmd(
        nc, in_maps, core_ids=list(range(_N)), trace=True
    )
    if res.exec_time_ns is not None:
        print(f"HW exec time: {res.exec_time_ns} ns")

    # out_dev[blk, 8*j + q%8, (q//8)*64 + c] holds pixel pix = 63*(16*blk+j)+q
    t = _PIX // _PIX_T
    q = _PIX % _PIX_T
    blk = t // _GT
    j = t % _GT
    row = 8 * j + q % 8
    col0 = (q // 8) * 64
    out = np.empty((_N, _C, _H, _W), np.float32)
    for n in range(_N):
        od = res.results[n]["out"]  # [NBLK, 128, 512]
        oc = od[
            blk[:, None], row[:, None], col0[:, None] + np.arange(_C)[None, :]
        ]  # [HWPIX, C]
        out[n] = oc.T.reshape(_C, _H, _W)
    return out

